# revision 12
# baseline (speedup 1.0000x reference)
"""ArcFace inner-product loss kernel for one TRN2 chip (8 NeuronCores).

Model-parallel over the class dimension C (classic ArcFace sharding):
each core owns a contiguous shard of classes, streams its weight shard
from HBM in fp16, and computes SCALE * (feat_n @ w_n.T) for its shard.

Host wrapper responsibilities (sharding/layout/assembly):
  - L2-normalize feat and weight rows, cast to fp16, pre-transpose into
    the [contraction-on-partitions] layout the TensorEngine needs.
  - Gather per-core output shards into the full [B, C] matrix.
  - Apply the ArcFace margin at the B (row, label) entries (the
    marginal-logits matrix differs from SCALE*cos in only B elements).

Outputs match reference.py: (marginal_logits, SCALE*cos, weights).
"""
import math

import numpy as np

import concourse.bass as bass
import concourse.tile as tile
from concourse import bacc, mybir
from concourse.bass_utils import run_bass_kernel_spmd

# Problem shape (hardcoded per harness contract).
B, D, C = 256, 512, 100000
NCORES = 8
CS = C // NCORES            # 12500 classes per core
CHUNK = 512                 # classes per matmul chunk (one PSUM bank)
NCHUNK = 25
CP = CHUNK * NCHUNK         # 12800 padded classes per core
KT = D // 128               # 4 contraction tiles
MT = B // 128               # 2 output row tiles
GROUP = 1                   # chunks per input DMA (512 KB fp16)
NGROUP = NCHUNK // GROUP

SCALE = 30.0
MARGIN = 0.5
THRESH = -math.cos(MARGIN)
SIN_M = math.sin(MARGIN)

_NC_CACHE = None


def _build():
    """Build + compile the per-core Bass graph (same NEFF on all 8 cores)."""
    nc = bacc.Bacc("TRN2", target_bir_lowering=False, debug=False,
                   enable_asserts=True, num_devices=NCORES)

    # nft[p, (t*MT+m)*128 + b] = SCALE * feat_n[m*128+b, t*128+p]
    nft = nc.dram_tensor("nft", [128, KT * MT * 128], mybir.dt.float16,
                         kind="ExternalInput").ap()
    # w[g, p, (c5*KT + t)*CHUNK + c] = w_n_shard[(g*GROUP+c5)*CHUNK+c, t*128+p]
    w = nc.dram_tensor("w", [NGROUP, 128, GROUP * KT * CHUNK], mybir.dt.float16,
                       kind="ExternalInput").ap()
    # out[ch, m, p, c] = SCALE * cos[m*128+p, ch*CHUNK+c]
    out = nc.dram_tensor("out", [NCHUNK, MT, 128, CHUNK], mybir.dt.float16,
                         kind="ExternalOutput").ap()

    with tile.TileContext(nc) as tc:
        with tc.tile_pool(name="const", bufs=1) as cpool, \
             tc.tile_pool(name="wpool", bufs=8) as wpool, \
             tc.tile_pool(name="opool", bufs=6) as opool, \
             tc.tile_pool(name="psum", bufs=4, space="PSUM") as psum:

            nft_sb = cpool.tile([128, KT * MT * 128], mybir.dt.float16, tag="nft")
            nc.sync.dma_start(nft_sb[:], nft[:])

            for g in range(NGROUP):
                wt = wpool.tile([128, GROUP * KT * CHUNK], mybir.dt.float16,
                                tag="wt")
                nc.sync.dma_start(wt[:], w[g])

                for c5 in range(GROUP):
                    ch = g * GROUP + c5
                    osb = opool.tile([128, MT * CHUNK], mybir.dt.float16,
                                     tag="osb")
                    for m in range(MT):
                        po = psum.tile([128, CHUNK], mybir.dt.float32,
                                       tag=f"po{m}")
                        for t in range(KT):
                            st = nft_sb[:, (t * MT + m) * 128:
                                        (t * MT + m + 1) * 128]
                            mv = wt[:, (c5 * KT + t) * CHUNK:
                                    (c5 * KT + t + 1) * CHUNK]
                            nc.tensor.matmul(po[:], st, mv,
                                             start=(t == 0), stop=(t == KT - 1))
                        dst = osb[:, m * CHUNK:(m + 1) * CHUNK]
                        if m == 0:
                            nc.vector.tensor_copy(dst, po[:])
                        else:
                            nc.scalar.copy(dst, po[:])
                    # one merged output DMA per chunk on the ACT HWDGE ring
                    dview = out[ch].rearrange("m p c -> p m c")
                    sview = osb[:].rearrange("p (m c) -> p m c", m=MT)
                    nc.gpsimd.dma_start(dview, sview)

    nc.compile()
    return nc


def _get_nc():
    global _NC_CACHE
    if _NC_CACHE is None:
        _NC_CACHE = _build()
    return _NC_CACHE


def _prep_inputs(feat, weights):
    featn = feat / np.linalg.norm(feat, axis=1, keepdims=True)
    featn = (SCALE * featn).astype(np.float16)
    # [p, (t, m, b)] stationary layout
    nft = np.empty((128, KT * MT * 128), np.float16)
    for t in range(KT):
        for m in range(MT):
            blk = featn[m * 128:(m + 1) * 128, t * 128:(t + 1) * 128]
            nft[:, (t * MT + m) * 128:(t * MT + m + 1) * 128] = blk.T

    in_maps = []
    for i in range(NCORES):
        shard = weights[i * CS:(i + 1) * CS]             # [12500, 512]
        wn = shard / np.linalg.norm(shard, axis=1, keepdims=True)
        full = np.ones((KT, 128, CP), np.float16)        # pad classes with 1.0
        full[:, :, :CS] = wn.T.reshape(KT, 128, CS).astype(np.float16)
        # -> [g, p, (c5, t, c)]
        wdev = np.ascontiguousarray(
            full.reshape(KT, 128, NGROUP, GROUP, CHUNK)
                .transpose(2, 1, 3, 0, 4)
                .reshape(NGROUP, 128, GROUP * KT * CHUNK))
        in_maps.append({"nft": nft, "w": wdev})
    return in_maps


def run(feat, weights, label, trace=False, trace_kwargs=None):
    """Full computation; returns ((marginal, scaled, weights), BassKernelResults)."""
    feat = np.asarray(feat, dtype=np.float32)
    weights = np.asarray(weights, dtype=np.float32)
    label = np.asarray(label)

    in_maps = _prep_inputs(feat, weights)
    nc = _get_nc()
    kw = {}
    if trace:
        kw["trace"] = True
        if trace_kwargs:
            kw.update(trace_kwargs)
    res = run_bass_kernel_spmd(nc, in_maps, core_ids=list(range(NCORES)), **kw)

    shards = []
    for i in range(NCORES):
        o = res.results[i]["out"]                        # [25, 2, 128, 512]
        o = o.astype(np.float32).transpose(1, 2, 0, 3).reshape(B, CP)[:, :CS]
        shards.append(o)
    scaled = np.ascontiguousarray(np.concatenate(shards, axis=1))  # 30*cos

    marginal = scaled.copy()
    rows = np.arange(B)
    lab = label.astype(np.int64)
    cos_t = np.clip(scaled[rows, lab] / SCALE, -1.0, 1.0)
    cond = cos_t > THRESH
    val = np.where(cond,
                   SCALE * np.cos(np.arccos(cos_t) + MARGIN),
                   SCALE * (cos_t - MARGIN * SIN_M))
    marginal[rows, lab] = val.astype(np.float32)

    return (marginal, scaled, weights), res


def kernel(feat, weights, label):
    outs, _ = run(feat, weights, label)
    return outs


# revision 16
# speedup vs baseline: 1.0210x; 1.0210x over previous
"""ArcFace inner-product loss kernel for one TRN2 chip (8 NeuronCores).

Model-parallel over the class dimension C (classic ArcFace sharding):
each core owns a contiguous shard of classes, streams its weight shard
from HBM in fp16, and computes SCALE * (feat_n @ w_n.T) for its shard.

Host wrapper responsibilities (sharding/layout/assembly):
  - L2-normalize feat and weight rows, cast to fp16, pre-transpose into
    the [contraction-on-partitions] layout the TensorEngine needs.
  - Gather per-core output shards into the full [B, C] matrix.
  - Apply the ArcFace margin at the B (row, label) entries (the
    marginal-logits matrix differs from SCALE*cos in only B elements).

Outputs match reference.py: (marginal_logits, SCALE*cos, weights).
"""
import math

import numpy as np

import concourse.bass as bass
import concourse.tile as tile
from concourse import bacc, mybir
from concourse.bass_utils import run_bass_kernel_spmd

# Problem shape (hardcoded per harness contract).
B, D, C = 256, 512, 100000
NCORES = 8
CS = C // NCORES            # 12500 classes per core
CP = 12544                  # padded classes per core (98 x 128)
CHUNKS = [512] * 24 + [256]         # classes per matmul chunk (<= 1 PSUM bank)
COFF = [sum(CHUNKS[:i]) for i in range(len(CHUNKS))]
KT = D // 128               # 4 contraction tiles
MT = B // 128               # 2 output row tiles

SCALE = 30.0
MARGIN = 0.5
THRESH = -math.cos(MARGIN)
SIN_M = math.sin(MARGIN)

_NC_CACHE = None


def _build():
    """Build + compile the per-core Bass graph (same NEFF on all 8 cores)."""
    nc = bacc.Bacc("TRN2", target_bir_lowering=False, debug=False,
                   enable_asserts=True, num_devices=NCORES)

    # nft[p, (t*MT+m)*128 + b] = SCALE * feat_n[m*128+b, t*128+p]
    nft = nc.dram_tensor("nft", [128, KT * MT * 128], mybir.dt.float16,
                         kind="ExternalInput").ap()
    # w[p, KT*COFF[ch] + (t*csize + c)] = w_n_shard[COFF[ch]+c, t*128+p]
    w = nc.dram_tensor("w", [128, KT * CP], mybir.dt.float16,
                       kind="ExternalInput").ap()
    # out[m, p, COFF[ch]+c] = SCALE * cos[m*128+p, COFF[ch]+c]
    out = nc.dram_tensor("out", [MT, 128, CP], mybir.dt.float16,
                         kind="ExternalOutput").ap()

    with tile.TileContext(nc) as tc:
        with tc.tile_pool(name="const", bufs=1) as cpool, \
             tc.tile_pool(name="wpool", bufs=8) as wpool, \
             tc.tile_pool(name="opool", bufs=6) as opool, \
             tc.tile_pool(name="psum", bufs=4, space="PSUM") as psum:

            nft_sb = cpool.tile([128, KT * MT * 128], mybir.dt.float16, tag="nft")
            nc.sync.dma_start(nft_sb[:], nft[:])

            for ch, csize in enumerate(CHUNKS):
                off = COFF[ch]
                wt = wpool.tile([128, KT * 512], mybir.dt.float16, tag="wt")
                nc.sync.dma_start(wt[:, :KT * csize],
                                  w[:, KT * off:KT * (off + csize)])

                osb = opool.tile([128, MT * 512], mybir.dt.float16, tag="osb")
                for m in range(MT):
                    po = psum.tile([128, 512], mybir.dt.float32, tag=f"po{m}")
                    for t in range(KT):
                        st = nft_sb[:, (t * MT + m) * 128:(t * MT + m + 1) * 128]
                        mv = wt[:, t * csize:(t + 1) * csize]
                        nc.tensor.matmul(po[:, :csize], st, mv,
                                         start=(t == 0), stop=(t == KT - 1))
                    dst = osb[:, m * csize:(m + 1) * csize]
                    if m == 0:
                        nc.vector.tensor_copy(dst, po[:, :csize])
                    else:
                        nc.scalar.copy(dst, po[:, :csize])
                # one merged output DMA per chunk, issued from the idle
                # GPSIMD (SWDGE) ring so it never contends with input DMAs
                dview = out[:, :, off:off + csize].rearrange("m p c -> p m c")
                sview = osb[:, :MT * csize].rearrange("p (m c) -> p m c", m=MT)
                nc.gpsimd.dma_start(dview, sview)

    nc.compile()
    return nc


def _get_nc():
    global _NC_CACHE
    if _NC_CACHE is None:
        _NC_CACHE = _build()
    return _NC_CACHE


def _prep_inputs(feat, weights):
    featn = feat / np.linalg.norm(feat, axis=1, keepdims=True)
    featn = (SCALE * featn).astype(np.float16)
    # [p, (t, m, b)] stationary layout
    nft = np.empty((128, KT * MT * 128), np.float16)
    for t in range(KT):
        for m in range(MT):
            blk = featn[m * 128:(m + 1) * 128, t * 128:(t + 1) * 128]
            nft[:, (t * MT + m) * 128:(t * MT + m + 1) * 128] = blk.T

    in_maps = []
    for i in range(NCORES):
        shard = weights[i * CS:(i + 1) * CS]             # [12500, 512]
        wn = shard / np.linalg.norm(shard, axis=1, keepdims=True)
        full = np.ones((KT, 128, CP), np.float16)        # pad classes with 1.0
        full[:, :, :CS] = wn.T.reshape(KT, 128, CS).astype(np.float16)
        # -> [p, (ch, t, c)] with per-chunk (t, c) blocks of varying csize
        wdev = np.empty((128, KT * CP), np.float16)
        for ch, csize in enumerate(CHUNKS):
            off = COFF[ch]
            blk = full[:, :, off:off + csize]            # [t, p, c]
            wdev[:, KT * off:KT * (off + csize)] = (
                blk.transpose(1, 0, 2).reshape(128, KT * csize))
        in_maps.append({"nft": nft, "w": wdev})
    return in_maps


def run(feat, weights, label, trace=False, trace_kwargs=None):
    """Full computation; returns ((marginal, scaled, weights), BassKernelResults)."""
    feat = np.asarray(feat, dtype=np.float32)
    weights = np.asarray(weights, dtype=np.float32)
    label = np.asarray(label)

    in_maps = _prep_inputs(feat, weights)
    nc = _get_nc()
    kw = {}
    if trace:
        kw["trace"] = True
        if trace_kwargs:
            kw.update(trace_kwargs)
    res = run_bass_kernel_spmd(nc, in_maps, core_ids=list(range(NCORES)), **kw)

    shards = []
    for i in range(NCORES):
        o = res.results[i]["out"]                        # [2, 128, CP]
        o = o.astype(np.float32).reshape(B, CP)[:, :CS]
        shards.append(o)
    scaled = np.ascontiguousarray(np.concatenate(shards, axis=1))  # 30*cos

    marginal = scaled.copy()
    rows = np.arange(B)
    lab = label.astype(np.int64)
    cos_t = np.clip(scaled[rows, lab] / SCALE, -1.0, 1.0)
    cond = cos_t > THRESH
    val = np.where(cond,
                   SCALE * np.cos(np.arccos(cos_t) + MARGIN),
                   SCALE * (cos_t - MARGIN * SIN_M))
    marginal[rows, lab] = val.astype(np.float32)

    return (marginal, scaled, weights), res


def kernel(feat, weights, label):
    outs, _ = run(feat, weights, label)
    return outs


# revision 17
# speedup vs baseline: 1.0606x; 1.0387x over previous
"""ArcFace inner-product loss kernel for one TRN2 chip (8 NeuronCores).

Model-parallel over the class dimension C (classic ArcFace sharding):
each core owns a contiguous shard of classes, streams its weight shard
from HBM in fp16, and computes SCALE * (feat_n @ w_n.T) for its shard.

Host wrapper responsibilities (sharding/layout/assembly):
  - L2-normalize feat and weight rows, cast to fp16, pre-transpose into
    the [contraction-on-partitions] layout the TensorEngine needs.
  - Gather per-core output shards into the full [B, C] matrix.
  - Apply the ArcFace margin at the B (row, label) entries (the
    marginal-logits matrix differs from SCALE*cos in only B elements).

Outputs match reference.py: (marginal_logits, SCALE*cos, weights).
"""
import math

import numpy as np

import concourse.bass as bass
import concourse.tile as tile
from concourse import bacc, mybir
from concourse.bass_utils import run_bass_kernel_spmd

# Problem shape (hardcoded per harness contract).
B, D, C = 256, 512, 100000
NCORES = 8
CS = C // NCORES            # 12500 classes per core
CP = 12544                  # padded classes per core (98 x 128)
# classes per matmul chunk (<= 1 PSUM bank = 512 f32): tapered so the
# first matmuls start as soon as a small DMA lands, and the tail drains fast
CHUNKS = [128, 128, 256] + [512] * 22 + [256, 256, 256]
COFF = [sum(CHUNKS[:i]) for i in range(len(CHUNKS))]
assert sum(CHUNKS) == CP
KT = D // 128               # 4 contraction tiles
MT = B // 128               # 2 output row tiles

SCALE = 30.0
MARGIN = 0.5
THRESH = -math.cos(MARGIN)
SIN_M = math.sin(MARGIN)

_NC_CACHE = None


def _build():
    """Build + compile the per-core Bass graph (same NEFF on all 8 cores)."""
    nc = bacc.Bacc("TRN2", target_bir_lowering=False, debug=False,
                   enable_asserts=True, num_devices=NCORES)

    # nft[p, (t*MT+m)*128 + b] = SCALE * feat_n[m*128+b, t*128+p]
    nft = nc.dram_tensor("nft", [128, KT * MT * 128], mybir.dt.float16,
                         kind="ExternalInput").ap()
    # w[p, KT*COFF[ch] + (t*csize + c)] = w_n_shard[COFF[ch]+c, t*128+p]
    w = nc.dram_tensor("w", [128, KT * CP], mybir.dt.float16,
                       kind="ExternalInput").ap()
    # out[m, p, COFF[ch]+c] = SCALE * cos[m*128+p, COFF[ch]+c]
    out = nc.dram_tensor("out", [MT, 128, CP], mybir.dt.float16,
                         kind="ExternalOutput").ap()

    with tile.TileContext(nc) as tc:
        with tc.tile_pool(name="const", bufs=1) as cpool, \
             tc.tile_pool(name="wpool", bufs=8) as wpool, \
             tc.tile_pool(name="opool", bufs=6) as opool, \
             tc.tile_pool(name="psum", bufs=4, space="PSUM") as psum:

            nft_sb = cpool.tile([128, KT * MT * 128], mybir.dt.float16, tag="nft")
            nc.sync.dma_start(nft_sb[:], nft[:])

            for ch, csize in enumerate(CHUNKS):
                off = COFF[ch]
                wt = wpool.tile([128, KT * 512], mybir.dt.float16, tag="wt")
                nc.sync.dma_start(wt[:, :KT * csize],
                                  w[:, KT * off:KT * (off + csize)])

                osb = opool.tile([128, MT * 512], mybir.dt.float16, tag="osb")
                for m in range(MT):
                    po = psum.tile([128, 512], mybir.dt.float32, tag=f"po{m}")
                    for t in range(KT):
                        st = nft_sb[:, (t * MT + m) * 128:(t * MT + m + 1) * 128]
                        mv = wt[:, t * csize:(t + 1) * csize]
                        nc.tensor.matmul(po[:, :csize], st, mv,
                                         start=(t == 0), stop=(t == KT - 1))
                    dst = osb[:, m * csize:(m + 1) * csize]
                    if m == 0:
                        nc.vector.tensor_copy(dst, po[:, :csize])
                    else:
                        nc.scalar.copy(dst, po[:, :csize])
                # one merged output DMA per chunk, issued from the idle
                # GPSIMD (SWDGE) ring so it never contends with input DMAs
                dview = out[:, :, off:off + csize].rearrange("m p c -> p m c")
                sview = osb[:, :MT * csize].rearrange("p (m c) -> p m c", m=MT)
                nc.gpsimd.dma_start(dview, sview)

    nc.compile()
    return nc


def _get_nc():
    global _NC_CACHE
    if _NC_CACHE is None:
        _NC_CACHE = _build()
    return _NC_CACHE


def _prep_inputs(feat, weights):
    featn = feat / np.linalg.norm(feat, axis=1, keepdims=True)
    featn = (SCALE * featn).astype(np.float16)
    # [p, (t, m, b)] stationary layout
    nft = np.empty((128, KT * MT * 128), np.float16)
    for t in range(KT):
        for m in range(MT):
            blk = featn[m * 128:(m + 1) * 128, t * 128:(t + 1) * 128]
            nft[:, (t * MT + m) * 128:(t * MT + m + 1) * 128] = blk.T

    in_maps = []
    for i in range(NCORES):
        shard = weights[i * CS:(i + 1) * CS]             # [12500, 512]
        wn = shard / np.linalg.norm(shard, axis=1, keepdims=True)
        full = np.ones((KT, 128, CP), np.float16)        # pad classes with 1.0
        full[:, :, :CS] = wn.T.reshape(KT, 128, CS).astype(np.float16)
        # -> [p, (ch, t, c)] with per-chunk (t, c) blocks of varying csize
        wdev = np.empty((128, KT * CP), np.float16)
        for ch, csize in enumerate(CHUNKS):
            off = COFF[ch]
            blk = full[:, :, off:off + csize]            # [t, p, c]
            wdev[:, KT * off:KT * (off + csize)] = (
                blk.transpose(1, 0, 2).reshape(128, KT * csize))
        in_maps.append({"nft": nft, "w": wdev})
    return in_maps


def run(feat, weights, label, trace=False, trace_kwargs=None):
    """Full computation; returns ((marginal, scaled, weights), BassKernelResults)."""
    feat = np.asarray(feat, dtype=np.float32)
    weights = np.asarray(weights, dtype=np.float32)
    label = np.asarray(label)

    in_maps = _prep_inputs(feat, weights)
    nc = _get_nc()
    kw = {}
    if trace:
        kw["trace"] = True
        if trace_kwargs:
            kw.update(trace_kwargs)
    res = run_bass_kernel_spmd(nc, in_maps, core_ids=list(range(NCORES)), **kw)

    shards = []
    for i in range(NCORES):
        o = res.results[i]["out"]                        # [2, 128, CP]
        o = o.astype(np.float32).reshape(B, CP)[:, :CS]
        shards.append(o)
    scaled = np.ascontiguousarray(np.concatenate(shards, axis=1))  # 30*cos

    marginal = scaled.copy()
    rows = np.arange(B)
    lab = label.astype(np.int64)
    cos_t = np.clip(scaled[rows, lab] / SCALE, -1.0, 1.0)
    cond = cos_t > THRESH
    val = np.where(cond,
                   SCALE * np.cos(np.arccos(cos_t) + MARGIN),
                   SCALE * (cos_t - MARGIN * SIN_M))
    marginal[rows, lab] = val.astype(np.float32)

    return (marginal, scaled, weights), res


def kernel(feat, weights, label):
    outs, _ = run(feat, weights, label)
    return outs


# revision 18
# speedup vs baseline: 1.1153x; 1.0516x over previous
"""ArcFace inner-product loss kernel for one TRN2 chip (8 NeuronCores).

Model-parallel over the class dimension C (classic ArcFace sharding):
each core owns a contiguous shard of classes, streams its weight shard
from HBM in fp16, and computes SCALE * (feat_n @ w_n.T) for its shard.

Host wrapper responsibilities (sharding/layout/assembly):
  - L2-normalize feat and weight rows, cast to fp16, pre-transpose into
    the [contraction-on-partitions] layout the TensorEngine needs.
  - Gather per-core output shards into the full [B, C] matrix.
  - Apply the ArcFace margin at the B (row, label) entries (the
    marginal-logits matrix differs from SCALE*cos in only B elements).

Outputs match reference.py: (marginal_logits, SCALE*cos, weights).
"""
import math

import numpy as np

import concourse.bass as bass
import concourse.tile as tile
from concourse import bacc, mybir
from concourse.bass_utils import run_bass_kernel_spmd

# Problem shape (hardcoded per harness contract).
B, D, C = 256, 512, 100000
NCORES = 8
CS = C // NCORES            # 12500 classes per core
CP = 12544                  # padded classes per core (98 x 128)
# classes per matmul chunk (<= 1 PSUM bank = 512 f32): tapered so the
# first matmuls start as soon as a small DMA lands, and the tail drains fast
CHUNKS = [128, 128, 256] + [512] * 22 + [256, 256, 256]
COFF = [sum(CHUNKS[:i]) for i in range(len(CHUNKS))]
assert sum(CHUNKS) == CP
KT = D // 128               # 4 contraction tiles
MT = B // 128               # 2 output row tiles

SCALE = 30.0
MARGIN = 0.5
THRESH = -math.cos(MARGIN)
SIN_M = math.sin(MARGIN)

_NC_CACHE = None


def _build():
    """Build + compile the per-core Bass graph (same NEFF on all 8 cores)."""
    nc = bacc.Bacc("TRN2", target_bir_lowering=False, debug=False,
                   enable_asserts=True, num_devices=NCORES)

    # nft[p, (t*MT+m)*128 + b] = SCALE * feat_n[m*128+b, t*128+p]
    nft = nc.dram_tensor("nft", [128, KT * MT * 128], mybir.dt.float16,
                         kind="ExternalInput").ap()
    # w[p, KT*COFF[ch] + (t*csize + c)] = w_n_shard[COFF[ch]+c, t*128+p]
    w = nc.dram_tensor("w", [128, KT * CP], mybir.dt.float16,
                       kind="ExternalInput").ap()
    # out[p, MT*COFF[ch] + m*csize + c] = SCALE * cos[m*128+p, COFF[ch]+c]
    out = nc.dram_tensor("out", [128, MT * CP], mybir.dt.float16,
                         kind="ExternalOutput").ap()

    with tile.TileContext(nc) as tc:
        with tc.tile_pool(name="const", bufs=1) as cpool, \
             tc.tile_pool(name="wpool", bufs=8) as wpool, \
             tc.tile_pool(name="opool", bufs=6) as opool, \
             tc.tile_pool(name="psum", bufs=4, space="PSUM") as psum:

            nft_sb = cpool.tile([128, KT * MT * 128], mybir.dt.float16, tag="nft")
            nc.sync.dma_start(nft_sb[:], nft[:])

            for ch, csize in enumerate(CHUNKS):
                off = COFF[ch]
                wt = wpool.tile([128, KT * 512], mybir.dt.float16, tag="wt")
                nc.sync.dma_start(wt[:, :KT * csize],
                                  w[:, KT * off:KT * (off + csize)])

                osb = opool.tile([128, MT * 512], mybir.dt.float16, tag="osb")
                for m in range(MT):
                    po = psum.tile([128, 512], mybir.dt.float32, tag=f"po{m}")
                    for t in range(KT):
                        st = nft_sb[:, (t * MT + m) * 128:(t * MT + m + 1) * 128]
                        mv = wt[:, t * csize:(t + 1) * csize]
                        nc.tensor.matmul(po[:, :csize], st, mv,
                                         start=(t == 0), stop=(t == KT - 1))
                    dst = osb[:, m * csize:(m + 1) * csize]
                    if m == 0:
                        nc.vector.tensor_copy(dst, po[:, :csize])
                    else:
                        nc.scalar.copy(dst, po[:, :csize])
                # one merged, fully contiguous output DMA per chunk, issued
                # from the idle GPSIMD (SWDGE) ring so it never contends with
                # the input stream
                nc.gpsimd.dma_start(out[:, MT * off:MT * (off + csize)],
                                    osb[:, :MT * csize])

    nc.compile()
    return nc


def _get_nc():
    global _NC_CACHE
    if _NC_CACHE is None:
        _NC_CACHE = _build()
    return _NC_CACHE


def _prep_inputs(feat, weights):
    featn = feat / np.linalg.norm(feat, axis=1, keepdims=True)
    featn = (SCALE * featn).astype(np.float16)
    # [p, (t, m, b)] stationary layout
    nft = np.empty((128, KT * MT * 128), np.float16)
    for t in range(KT):
        for m in range(MT):
            blk = featn[m * 128:(m + 1) * 128, t * 128:(t + 1) * 128]
            nft[:, (t * MT + m) * 128:(t * MT + m + 1) * 128] = blk.T

    in_maps = []
    for i in range(NCORES):
        shard = weights[i * CS:(i + 1) * CS]             # [12500, 512]
        wn = shard / np.linalg.norm(shard, axis=1, keepdims=True)
        full = np.ones((KT, 128, CP), np.float16)        # pad classes with 1.0
        full[:, :, :CS] = wn.T.reshape(KT, 128, CS).astype(np.float16)
        # -> [p, (ch, t, c)] with per-chunk (t, c) blocks of varying csize
        wdev = np.empty((128, KT * CP), np.float16)
        for ch, csize in enumerate(CHUNKS):
            off = COFF[ch]
            blk = full[:, :, off:off + csize]            # [t, p, c]
            wdev[:, KT * off:KT * (off + csize)] = (
                blk.transpose(1, 0, 2).reshape(128, KT * csize))
        in_maps.append({"nft": nft, "w": wdev})
    return in_maps


def run(feat, weights, label, trace=False, trace_kwargs=None):
    """Full computation; returns ((marginal, scaled, weights), BassKernelResults)."""
    feat = np.asarray(feat, dtype=np.float32)
    weights = np.asarray(weights, dtype=np.float32)
    label = np.asarray(label)

    in_maps = _prep_inputs(feat, weights)
    nc = _get_nc()
    kw = {}
    if trace:
        kw["trace"] = True
        if trace_kwargs:
            kw.update(trace_kwargs)
    res = run_bass_kernel_spmd(nc, in_maps, core_ids=list(range(NCORES)), **kw)

    shards = []
    for i in range(NCORES):
        o = res.results[i]["out"].astype(np.float32)     # [128, MT*CP]
        full = np.empty((B, CP), np.float32)
        for ch, csize in enumerate(CHUNKS):
            off = COFF[ch]
            blk = o[:, MT * off:MT * (off + csize)].reshape(128, MT, csize)
            full[:, off:off + csize] = blk.transpose(1, 0, 2).reshape(B, csize)
        shards.append(full[:, :CS])
    scaled = np.ascontiguousarray(np.concatenate(shards, axis=1))  # 30*cos

    marginal = scaled.copy()
    rows = np.arange(B)
    lab = label.astype(np.int64)
    cos_t = np.clip(scaled[rows, lab] / SCALE, -1.0, 1.0)
    cond = cos_t > THRESH
    val = np.where(cond,
                   SCALE * np.cos(np.arccos(cos_t) + MARGIN),
                   SCALE * (cos_t - MARGIN * SIN_M))
    marginal[rows, lab] = val.astype(np.float32)

    return (marginal, scaled, weights), res


def kernel(feat, weights, label):
    outs, _ = run(feat, weights, label)
    return outs


# revision 20
# speedup vs baseline: 1.1514x; 1.0324x over previous
"""ArcFace inner-product loss kernel for one TRN2 chip (8 NeuronCores).

Model-parallel over the class dimension C (classic ArcFace sharding):
each core owns a contiguous shard of classes, streams its weight shard
from HBM in fp16, and computes SCALE * (feat_n @ w_n.T) for its shard.

Host wrapper responsibilities (sharding/layout/assembly):
  - L2-normalize feat and weight rows, cast to fp16, pre-transpose into
    the [contraction-on-partitions] layout the TensorEngine needs.
  - Gather per-core output shards into the full [B, C] matrix.
  - Apply the ArcFace margin at the B (row, label) entries (the
    marginal-logits matrix differs from SCALE*cos in only B elements).

Outputs match reference.py: (marginal_logits, SCALE*cos, weights).
"""
import math

import numpy as np

import concourse.bass as bass
import concourse.tile as tile
from concourse import bacc, mybir
from concourse.bass_utils import run_bass_kernel_spmd

# Problem shape (hardcoded per harness contract).
B, D, C = 256, 512, 100000
NCORES = 8
CS = C // NCORES            # 12500 classes per core
CP = 12544                  # padded classes per core (98 x 128)
# classes per matmul chunk (<= 1 PSUM bank = 512 f32): tapered so the
# first matmuls start as soon as a small DMA lands, and the tail drains fast
CHUNKS = [128, 128, 256] + [512] * 22 + [256, 256, 256]
COFF = [sum(CHUNKS[:i]) for i in range(len(CHUNKS))]
assert sum(CHUNKS) == CP
# consecutive chunks sharing one input DMA + one output DMA (1 MB / 512 KB
# in the steady state); singles at the edges preserve the taper
DGROUPS = [[0], [1], [2]] + [[i, i + 1] for i in range(3, 25, 2)] + \
          [[25], [26], [27]]
KT = D // 128               # 4 contraction tiles
MT = B // 128               # 2 output row tiles

SCALE = 30.0
MARGIN = 0.5
THRESH = -math.cos(MARGIN)
SIN_M = math.sin(MARGIN)

_NC_CACHE = None


def _build():
    """Build + compile the per-core Bass graph (same NEFF on all 8 cores)."""
    nc = bacc.Bacc("TRN2", target_bir_lowering=False, debug=False,
                   enable_asserts=True, num_devices=NCORES)

    # nft[p, (t*MT+m)*128 + b] = SCALE * feat_n[m*128+b, t*128+p]
    nft = nc.dram_tensor("nft", [128, KT * MT * 128], mybir.dt.float16,
                         kind="ExternalInput").ap()
    # w[p, KT*COFF[ch] + (t*csize + c)] = w_n_shard[COFF[ch]+c, t*128+p]
    w = nc.dram_tensor("w", [128, KT * CP], mybir.dt.float16,
                       kind="ExternalInput").ap()
    # out[p, MT*COFF[ch] + m*csize + c] = SCALE * cos[m*128+p, COFF[ch]+c]
    out = nc.dram_tensor("out", [128, MT * CP], mybir.dt.float16,
                         kind="ExternalOutput").ap()

    with tile.TileContext(nc) as tc:
        with tc.tile_pool(name="const", bufs=1) as cpool, \
             tc.tile_pool(name="wpool", bufs=8) as wpool, \
             tc.tile_pool(name="opool", bufs=6) as opool, \
             tc.tile_pool(name="psum", bufs=4, space="PSUM") as psum:

            nft_sb = cpool.tile([128, KT * MT * 128], mybir.dt.float16, tag="nft")
            nc.sync.dma_start(nft_sb[:], nft[:])

            for grp in DGROUPS:
                goff = COFF[grp[0]]
                gsize = sum(CHUNKS[ch] for ch in grp)
                wt = wpool.tile([128, KT * 1024], mybir.dt.float16, tag="wt")
                nc.sync.dma_start(wt[:, :KT * gsize],
                                  w[:, KT * goff:KT * (goff + gsize)])

                osb = opool.tile([128, MT * 1024], mybir.dt.float16, tag="osb")
                for ch in grp:
                    csize = CHUNKS[ch]
                    loff = COFF[ch] - goff
                    wb = wt[:, KT * loff:KT * (loff + csize)]
                    ob = osb[:, MT * loff:MT * (loff + csize)]
                    for m in range(MT):
                        po = psum.tile([128, 512], mybir.dt.float32,
                                       tag=f"po{m}")
                        for t in range(KT):
                            st = nft_sb[:, (t * MT + m) * 128:
                                        (t * MT + m + 1) * 128]
                            mv = wb[:, t * csize:(t + 1) * csize]
                            nc.tensor.matmul(po[:, :csize], st, mv,
                                             start=(t == 0), stop=(t == KT - 1))
                        dst = ob[:, m * csize:(m + 1) * csize]
                        if m == 0:
                            nc.vector.tensor_copy(dst, po[:, :csize])
                        else:
                            nc.scalar.copy(dst, po[:, :csize])
                # one merged, fully contiguous output DMA per group, issued
                # from the idle GPSIMD (SWDGE) ring so it never contends with
                # the input stream
                nc.gpsimd.dma_start(out[:, MT * goff:MT * (goff + gsize)],
                                    osb[:, :MT * gsize])

    nc.compile()
    return nc


def _get_nc():
    global _NC_CACHE
    if _NC_CACHE is None:
        _NC_CACHE = _build()
    return _NC_CACHE


def _prep_inputs(feat, weights):
    featn = feat / np.linalg.norm(feat, axis=1, keepdims=True)
    featn = (SCALE * featn).astype(np.float16)
    # [p, (t, m, b)] stationary layout
    nft = np.empty((128, KT * MT * 128), np.float16)
    for t in range(KT):
        for m in range(MT):
            blk = featn[m * 128:(m + 1) * 128, t * 128:(t + 1) * 128]
            nft[:, (t * MT + m) * 128:(t * MT + m + 1) * 128] = blk.T

    in_maps = []
    for i in range(NCORES):
        shard = weights[i * CS:(i + 1) * CS]             # [12500, 512]
        wn = shard / np.linalg.norm(shard, axis=1, keepdims=True)
        full = np.ones((KT, 128, CP), np.float16)        # pad classes with 1.0
        full[:, :, :CS] = wn.T.reshape(KT, 128, CS).astype(np.float16)
        # -> [p, (ch, t, c)] with per-chunk (t, c) blocks of varying csize
        wdev = np.empty((128, KT * CP), np.float16)
        for ch, csize in enumerate(CHUNKS):
            off = COFF[ch]
            blk = full[:, :, off:off + csize]            # [t, p, c]
            wdev[:, KT * off:KT * (off + csize)] = (
                blk.transpose(1, 0, 2).reshape(128, KT * csize))
        in_maps.append({"nft": nft, "w": wdev})
    return in_maps


def run(feat, weights, label, trace=False, trace_kwargs=None):
    """Full computation; returns ((marginal, scaled, weights), BassKernelResults)."""
    feat = np.asarray(feat, dtype=np.float32)
    weights = np.asarray(weights, dtype=np.float32)
    label = np.asarray(label)

    in_maps = _prep_inputs(feat, weights)
    nc = _get_nc()
    kw = {}
    if trace:
        kw["trace"] = True
        if trace_kwargs:
            kw.update(trace_kwargs)
    res = run_bass_kernel_spmd(nc, in_maps, core_ids=list(range(NCORES)), **kw)

    shards = []
    for i in range(NCORES):
        o = res.results[i]["out"].astype(np.float32)     # [128, MT*CP]
        full = np.empty((B, CP), np.float32)
        for ch, csize in enumerate(CHUNKS):
            off = COFF[ch]
            blk = o[:, MT * off:MT * (off + csize)].reshape(128, MT, csize)
            full[:, off:off + csize] = blk.transpose(1, 0, 2).reshape(B, csize)
        shards.append(full[:, :CS])
    scaled = np.ascontiguousarray(np.concatenate(shards, axis=1))  # 30*cos

    marginal = scaled.copy()
    rows = np.arange(B)
    lab = label.astype(np.int64)
    cos_t = np.clip(scaled[rows, lab] / SCALE, -1.0, 1.0)
    cond = cos_t > THRESH
    val = np.where(cond,
                   SCALE * np.cos(np.arccos(cos_t) + MARGIN),
                   SCALE * (cos_t - MARGIN * SIN_M))
    marginal[rows, lab] = val.astype(np.float32)

    return (marginal, scaled, weights), res


def kernel(feat, weights, label):
    outs, _ = run(feat, weights, label)
    return outs
